# revision 46
# baseline (speedup 1.0000x reference)
"""Multi-head attention (b=4, l=2048, d=1024, 16 heads) on 8 TRN2 NeuronCores.

Sharding: data parallel over the 4 batches x tensor parallel over 2 head
groups (8 heads each). core = 2*batch + head_group. Each core computes its
batch's attention for its 8 heads plus the partial W_o projection
(row-parallel); the host sums the two partials per batch and adds b_o.
No on-chip collectives needed.

Per-core layout (everything transposed so features sit on partitions):
  xT  [1024, 2048]; Q^T,K^T [512, 2048]; V in [m, o] layout.
  Per head pair (sharing an o-partition tile, rows 0-63 / 64-127):
    S^T = K Q^T for both heads into one [128, 1024] PSUM tile
    -> one exp on ScalarE -> P^T bf16
    -> A^T accumulated via V_aug (64 V cols + ones col -> denom in row 64)
  Softmax denominators are normalized immediately per (pair, lc, head):
  reciprocal straight off PSUM row 64, gpsimd partition-broadcast, one DVE
  multiply -- no DMA round trips and no end-of-pair burst.
  Y_partial = A^T.T-contract @ WoT -> [2048, 1024] fp32.

All DRAM operands are host-pretiled so every DMA is contiguous per
partition (W_q/W_k additionally ot-major so the first projection group's
slab is one small contiguous load); startup DMAs are split fine-grained
across the SP/ACT/Pool queues in first-use order so the first matmul
starts ~2.4us in.

The m-tile loop is software-pipelined: S^T for unit i+1 is emitted before
the exp-dependent A^T-accumulation of unit i, so the in-order PE queue
always holds exp-independent work while ScalarE runs.  Filler work (the
next pair's QK projection groups, the O-projection tiles) is spread one
or two matmuls per m-tile slot inside the attention stream -- whole-group
bursts only in pair 0, whose own groups have just-in-time deadlines; the
last chunk normalizes in 128-column slices so the four tail O-projections
unlock one by one.  Everything is bf16/f32: on this hardware fp8
DoubleRow costs the same per instruction as bf16 (2x MACs only via
K-packing), and any fp8 quantization in the value path adds ~2.5% RMS to
the output -- over the error budget -- so the bf16 stream-cycle floor
(~327us busy PE) is the binding constraint and scheduling is everything.
"""

import os

import numpy as np

B = 4
L = 2048
D = 1024
P = 128
NH = 8          # heads per core
DH = 64
O = NH * DH     # 512 qkv dims per core
DT = D // P     # 8 d-tiles
OT = O // P     # 4 o-tiles (= head pairs)
MT = L // P     # 16 m-tiles
LC4 = L // 512  # 4 l-chunks of 512

_cache = {}


def _build():
    import concourse.tile as tile
    from concourse import bacc, mybir

    nc = bacc.Bacc("TRN2", target_bir_lowering=False, debug=False)
    bf = mybir.dt.bfloat16
    f32 = mybir.dt.float32
    Exp = mybir.ActivationFunctionType.Exp

    # host-pretiled: row p holds the concatenation over tiles (see
    # make_in_maps) so per-partition DRAM runs are large and contiguous.
    xT = nc.dram_tensor("xT", [P, DT * L], bf, kind="ExternalInput").ap()
    # wq/wk are host-pretiled ot-major ([p, ot, dt, 128]) so the ot=0 slab
    # the first projection group needs is one contiguous 2KB-per-partition
    # DMA; wv/wo stay dt-major like the baseline.
    wqT = nc.dram_tensor("wqT", [P, DT * O], bf, kind="ExternalInput").ap()
    wkT = nc.dram_tensor("wkT", [P, DT * O], bf, kind="ExternalInput").ap()
    wvT = nc.dram_tensor("wvT", [P, DT * O], bf, kind="ExternalInput").ap()
    woT = nc.dram_tensor("woT", [P, OT * D], bf, kind="ExternalInput").ap()
    out = nc.dram_tensor("out", [P, MT * D], f32, kind="ExternalOutput").ap()
    xT3 = xT.rearrange("p (dt l) -> p dt l", l=L)
    wq4 = wqT.rearrange("p (ot dt c) -> p ot dt c", dt=DT, c=P)
    wk4 = wkT.rearrange("p (ot dt c) -> p ot dt c", dt=DT, c=P)
    out3 = out.rearrange("p (lt j) -> p lt j", j=D)

    with tile.TileContext(nc) as tc:
        with (
            tc.tile_pool(name="persist", bufs=1) as pp,
            tc.tile_pool(name="work", bufs=3) as wp,
            tc.tile_pool(name="psum", bufs=1, space="PSUM") as psp,
        ):
            xT_sb = pp.tile([P, DT, L], bf, name="xT_sb")
            wq_sb = pp.tile([P, OT, DT, P], bf, name="wq_sb")
            wk_sb = pp.tile([P, OT, DT, P], bf, name="wk_sb")
            wv_sb = pp.tile([P, DT, O], bf, name="wv_sb")
            wo_sb = pp.tile([P, OT, D], bf, name="wo_sb")
            qT_sb = pp.tile([P, OT, L], bf, name="qT_sb")
            kT_sb = pp.tile([P, OT, L], bf, name="kT_sb")
            vaug_sb = pp.tile([P, MT, NH * 65], bf, name="vaug_sb")
            at_sb = pp.tile([P, OT, L], bf, name="at_sb")

            # Fine-grained startup: the ot=0 slices of W_q/W_k plus the
            # first 512 columns of x are all the first two projection
            # groups need; load those first, then stream the rest in
            # first-use order across the three DMA-capable queues (SP,
            # Pool, ACT) so nothing serializes behind a bulk load.
            nc.gpsimd.dma_start(xT_sb[:, 0:4, 0:512], xT3[:, 0:4, 0:512])
            nc.scalar.dma_start(xT_sb[:, 4:8, 0:512], xT3[:, 4:8, 0:512])
            nc.sync.dma_start(wq_sb[:, 0, 0:4], wq4[:, 0, 0:4])
            nc.sync.dma_start(wq_sb[:, 0, 4:8], wq4[:, 0, 4:8])
            nc.sync.dma_start(wk_sb[:, 0], wk4[:, 0])
            nc.scalar.dma_start(xT_sb[:, :, 512:1024], xT3[:, :, 512:1024])
            nc.sync.dma_start(wv_sb[:], wvT[:].rearrange("p (dt o) -> p dt o", o=O))
            nc.sync.dma_start(wq_sb[:, 1:OT], wq4[:, 1:OT])
            nc.sync.dma_start(wk_sb[:, 1:OT], wk4[:, 1:OT])
            nc.gpsimd.dma_start(xT_sb[:, :, 1024:1536], xT3[:, :, 1024:1536])
            nc.gpsimd.dma_start(xT_sb[:, :, 1536:2048], xT3[:, :, 1536:2048])
            nc.sync.dma_start(wo_sb[:], woT[:].rearrange("p (ot j) -> p ot j", j=D))

            # ones columns for the softmax-denominator trick; V copies below
            # fill the first 64 columns of each head's 65-column block, so
            # only column 64 of each block actually needs the memset.
            for h in range(NH):
                nc.gpsimd.memset(vaug_sb[:, :, h * 65 + 64], 1.0)

            def proj_qk_group(w_sb, dst, ot, lc):
                ps = psp.tile([P, 512], f32, tag="mm512", bufs=2, name="ps_qk")
                for dt in range(DT):
                    nc.tensor.matmul(
                        ps[:],
                        w_sb[:, ot, dt, :],
                        xT_sb[:, dt, lc * 512:(lc + 1) * 512],
                        start=(dt == 0),
                        stop=(dt == DT - 1),
                    )
                nc.vector.tensor_copy(dst[:, ot, lc * 512:(lc + 1) * 512], ps[:])

            def proj_v(mt):
                ps = psp.tile([P, 512], f32, tag="mm512", bufs=2, name="ps_v")
                for dt in range(DT):
                    nc.tensor.matmul(
                        ps[:],
                        xT_sb[:, dt, mt * P:(mt + 1) * P],
                        wv_sb[:, dt, :],
                        start=(dt == 0),
                        stop=(dt == DT - 1),
                    )
                for h in range(NH):
                    nc.gpsimd.tensor_copy(
                        vaug_sb[:, mt, h * 65:h * 65 + 64],
                        ps[:, h * DH:(h + 1) * DH],
                    )

            def proj_o(lt, tail=False):
                ob = wp.tile([P, 1024], f32, tag="ob", bufs=3, name="ob")
                for jc in range(2):
                    ps = psp.tile([P, 512], f32, tag="mm512", bufs=2, name="ps_o")
                    for ot in range(OT):
                        nc.tensor.matmul(
                            ps[:],
                            at_sb[:, ot, lt * P:(lt + 1) * P],
                            wo_sb[:, ot, jc * 512:(jc + 1) * 512],
                            start=(ot == 0),
                            stop=(ot == OT - 1),
                        )
                    nc.gpsimd.tensor_copy(ob[:, jc * 512:(jc + 1) * 512], ps[:])
                    if tail:
                        # drain in quarters on both spare queues as soon as
                        # each copy lands, instead of one big store.
                        for q, eng in ((0, nc.sync), (1, nc.gpsimd)):
                            qs = slice(jc * 512 + q * 256, jc * 512 + (q + 1) * 256)
                            eng.dma_start(out3[:, lt, qs], ob[:, qs])
                if not tail:
                    nc.sync.dma_start(out3[:, lt, :], ob[:])

            # Only the two groups the very first S^T matmul needs are
            # emitted upfront; everything else streams in as filler work
            # inside the attention mt loops.
            proj_qk_group(wq_sb, qT_sb, 0, 0)
            proj_qk_group(wk_sb, kT_sb, 0, 0)

            # ---- attention, one head pair at a time -----------------------
            # Filler emissions (projection groups / O-projection tiles) are
            # woven INSIDE the mt loops so the PE always has matmul work
            # during the ACT-bound exp stream.  Pair 0 uses whole-group
            # bursts (its own groups have hard deadlines and mixing bursts
            # with spread streams on one PSUM tag would deadlock the
            # in-order PE queue); pairs 1-3 spread their filler matmuls one
            # or two per m-tile so the PE never drains between bursts.
            def group_thunks(w_sb, dst, ot, lc):
                st = {}

                def mm(dt):
                    if dt == 0:
                        st["ps"] = psp.tile(
                            [P, 512], f32, tag="mm512", bufs=2, name="ps_qk"
                        )
                    nc.tensor.matmul(
                        st["ps"][:],
                        w_sb[:, ot, dt, :],
                        xT_sb[:, dt, lc * 512:(lc + 1) * 512],
                        start=(dt == 0),
                        stop=(dt == DT - 1),
                    )
                    if dt == DT - 1:
                        nc.vector.tensor_copy(
                            dst[:, ot, lc * 512:(lc + 1) * 512], st["ps"][:]
                        )

                return [lambda dt=dt: mm(dt) for dt in range(DT)]

            def proj_o_thunks(lt):
                st = {}

                def mm(j):
                    jc, ot = divmod(j, OT)
                    if j == 0:
                        st["ob"] = wp.tile([P, 1024], f32, tag="ob", bufs=3, name="ob")
                    if ot == 0:
                        st["ps"] = psp.tile(
                            [P, 512], f32, tag="mm512", bufs=2, name="ps_o"
                        )
                    nc.tensor.matmul(
                        st["ps"][:],
                        at_sb[:, ot, lt * P:(lt + 1) * P],
                        wo_sb[:, ot, jc * 512:(jc + 1) * 512],
                        start=(ot == 0),
                        stop=(ot == OT - 1),
                    )
                    if ot == OT - 1:
                        nc.vector.tensor_copy(
                            st["ob"][:, jc * 512:(jc + 1) * 512], st["ps"][:]
                        )
                        if jc == 1:
                            nc.sync.dma_start(out3[:, lt, :], st["ob"][:])

                return [lambda j=j: mm(j) for j in range(2 * OT)]

            def fillers_for(pair, lc):
                f = {}
                if pair == 0:
                    # pair 0's own remaining groups: kT m-groups just in
                    # time (S at m-tile mt reads kT group mt//4), qT for
                    # the next l-chunk midway through the previous one.
                    if lc == 0:
                        # S for m-tile 4g is emitted one unit early (software
                        # pipelining), so group g must be emitted by slot
                        # 4g-2, not 4g-1.
                        for g in (1, 2, 3):
                            f.setdefault(4 * g - 2, []).append(
                                lambda g=g: proj_qk_group(wk_sb, kT_sb, 0, g))
                    if lc < 3:
                        f.setdefault(6, []).append(
                            lambda lc=lc: proj_qk_group(wq_sb, qT_sb, 0, lc + 1))
                stream = []
                if pair < OT - 1:
                    # next pair's 8 groups: two per l-chunk, k-groups first
                    nxt = pair + 1
                    order = [(wq_sb, qT_sb, 0), (wk_sb, kT_sb, 0),
                             (wk_sb, kT_sb, 1), (wk_sb, kT_sb, 2),
                             (wk_sb, kT_sb, 3), (wq_sb, qT_sb, 1),
                             (wq_sb, qT_sb, 2), (wq_sb, qT_sb, 3)]
                    for i in (2 * lc, 2 * lc + 1):
                        w_sb, dst, g = order[i]
                        if pair == 0:
                            f.setdefault(7 + 8 * (i % 2), []).append(
                                lambda w_sb=w_sb, dst=dst, g=g:
                                    proj_qk_group(w_sb, dst, nxt, g))
                        else:
                            stream.extend(group_thunks(w_sb, dst, nxt, g))
                if pair == OT - 1 and lc > 0:
                    # O-projection tiles unlocked one l-chunk back (the
                    # immediate normalize finishes each chunk's at rows as
                    # the chunk ends).  In the last chunk, hold one back to
                    # bridge the final normalize latency.
                    lts = range(4 * (lc - 1), 4 * lc - (1 if lc == LC4 - 1 else 0))
                    for lt in lts:
                        stream.extend(proj_o_thunks(lt))
                # distribute over slots 0..14 so the final group copy is
                # off the DVE queue before mt15's normalize chain needs it
                k = len(stream)
                n = MT - 1
                for i in range(n):
                    for th in stream[(k * i) // n:(k * (i + 1)) // n]:
                        f.setdefault(i, []).append(th)
                return f

            def normalize(pair, lc, av, rb, sl=None):
                """at[rb:rb+64, pair, lc] = av[0:64] * (1/av[64]) immediately
                off PSUM: DVE reciprocal -> Pool broadcast -> DVE multiply.
                sl selects a 128-column slice (tail pipelining); None = all."""
                if sl is None:
                    csl = slice(0, 512)
                    w = 512
                else:
                    csl = slice(sl * P, (sl + 1) * P)
                    w = P
                cols = slice(lc * 512 + csl.start, lc * 512 + csl.stop)
                rt = wp.tile([1, 512], f32, tag="rt", bufs=4, name="rt")
                nc.vector.reciprocal(rt[:, 0:w], av[64:65, csl])
                rbb = wp.tile([DH, 512], f32, tag="rbb", bufs=4, name="rbb")
                nc.gpsimd.partition_broadcast(rbb[:, 0:w], rt[:, 0:w])
                nc.vector.tensor_mul(
                    out=at_sb[rb:rb + DH, pair, cols],
                    in0=av[0:DH, csl],
                    in1=rbb[:, 0:w],
                )

            # The mt loop is software-pipelined: S^T for unit i+1 is
            # emitted BEFORE the exp-dependent AV of unit i, so the in-order
            # PE queue always has exp-independent work while ACT runs.
            units = [
                (pair, lc, mt)
                for pair in range(OT) for lc in range(LC4) for mt in range(MT)
            ]
            s_tiles = {}

            def emit_S(pair, lc, mt):
                cols = slice(lc * 512, (lc + 1) * 512)
                mcols = slice(mt * P, (mt + 1) * P)
                s = psp.tile([P, 1024], f32, tag="s", bufs=2, name="s")
                s_tiles[(pair, lc, mt)] = s
                nc.tensor.matmul(
                    s[:, 0:512],
                    kT_sb[0:DH, pair, mcols],
                    qT_sb[0:DH, pair, cols],
                    start=True, stop=True,
                )
                nc.tensor.matmul(
                    s[:, 512:1024],
                    kT_sb[DH:2 * DH, pair, mcols],
                    qT_sb[DH:2 * DH, pair, cols],
                    start=True, stop=True,
                )

            emit_S(*units[0])
            fill = {}
            av0 = av1 = None
            for ui, (pair, lc, mt) in enumerate(units):
                h0, h1 = 2 * pair, 2 * pair + 1
                if mt == 0:
                    fill = fillers_for(pair, lc)
                    av0 = psp.tile([P, 512], f32, tag="av", bufs=2, name="av0")
                    av1 = psp.tile([P, 512], f32, tag="av", bufs=2, name="av1")
                s = s_tiles.pop((pair, lc, mt))
                p = wp.tile([P, 1024], bf, tag="p", bufs=8, name="p")
                nc.scalar.activation(p[:], s[:], Exp, scale=0.125)
                if ui + 1 < len(units):
                    emit_S(*units[ui + 1])
                if pair == 0 and lc == 0:
                    proj_v(mt)
                nc.tensor.matmul(
                    av0[0:65, :],
                    vaug_sb[:, mt, h0 * 65:h0 * 65 + 65],
                    p[:, 0:512],
                    start=(mt == 0), stop=(mt == MT - 1),
                )
                nc.tensor.matmul(
                    av1[0:65, :],
                    vaug_sb[:, mt, h1 * 65:h1 * 65 + 65],
                    p[:, 512:1024],
                    start=(mt == 0), stop=(mt == MT - 1),
                )
                for fn in fill.get(mt, ()):
                    fn()
                if mt < MT - 1:
                    continue
                if pair == OT - 1 and lc == LC4 - 1:
                    # Final chunk: normalize in 128-column slices so each
                    # tail O-projection's at columns unlock as early as
                    # possible; proj_o(11) (needing only lc2's at) keeps
                    # the PE fed during the first slice's chain.
                    normalize(pair, lc, av0, 0, sl=0)
                    normalize(pair, lc, av1, DH, sl=0)
                    proj_o(11)
                    for sl in range(1, 4):
                        normalize(pair, lc, av0, 0, sl=sl)
                        normalize(pair, lc, av1, DH, sl=sl)
                        proj_o(11 + sl, tail=True)
                else:
                    normalize(pair, lc, av0, 0)
                    normalize(pair, lc, av1, DH)
            proj_o(15, tail=True)

    nc.compile()
    return nc


def get_nc():
    if "nc" not in _cache:
        _cache["nc"] = _build()
    return _cache["nc"]


def _pretile(a, p=P):
    """[T*p, F] -> [p, T*F] with row i holding concat over tiles t of a[t*p+i]."""
    t = a.shape[0] // p
    return np.ascontiguousarray(
        a.reshape(t, p, a.shape[1]).transpose(1, 0, 2).reshape(p, t * a.shape[1])
    )


def _pretile_ot(a, p=P):
    """[DT*p, OT*p] -> [p, OT*DT*p]: ot-major pretile so each ot's slab of
    W^T ([p partitions, DT, p] columns) is contiguous per partition."""
    dt = a.shape[0] // p
    ot = a.shape[1] // p
    # a[dt*p + i, ot*p + c] -> out[i, ((ot*DT + dt)*p) + c]
    return np.ascontiguousarray(
        a.reshape(dt, p, ot, p).transpose(1, 2, 0, 3).reshape(p, ot * dt * p)
    )


def make_in_maps(x, W_q, W_k, W_v, W_o):
    import ml_dtypes

    bf = ml_dtypes.bfloat16
    x = np.asarray(x, dtype=np.float32)
    W_q = np.asarray(W_q, dtype=np.float32)
    W_k = np.asarray(W_k, dtype=np.float32)
    W_v = np.asarray(W_v, dtype=np.float32)
    W_o = np.asarray(W_o, dtype=np.float32)

    in_maps = []
    for core in range(8):
        b, hg = divmod(core, 2)
        rows = slice(hg * O, (hg + 1) * O)
        in_maps.append({
            "xT": _pretile(np.ascontiguousarray(x[b].T)).astype(bf),
            "wqT": _pretile_ot(np.ascontiguousarray(W_q[rows].T)).astype(bf),
            "wkT": _pretile_ot(np.ascontiguousarray(W_k[rows].T)).astype(bf),
            "wvT": _pretile(np.ascontiguousarray(W_v[rows].T)).astype(bf),
            "woT": _pretile(np.ascontiguousarray(W_o[:, rows].T)).astype(bf),
        })
    return in_maps


def kernel(x, W_q, W_k, W_v, W_o, b_o):
    from concourse.bass_utils import run_bass_kernel_spmd

    nc = get_nc()
    in_maps = make_in_maps(x, W_q, W_k, W_v, W_o)
    trace = bool(int(os.environ.get("ATTN_TRACE", "0")))
    try:
        res = run_bass_kernel_spmd(nc, in_maps, core_ids=list(range(8)), trace=trace)
    except ModuleNotFoundError:
        # this container lacks the axon NTFF profile hook; rerun untraced
        trace = False
        res = run_bass_kernel_spmd(nc, in_maps, core_ids=list(range(8)), trace=False)
    if trace and res.exec_time_ns is not None:
        _cache["exec_time_ns"] = res.exec_time_ns
        _cache["mean_exec_time_ns"] = res.mean_exec_time_ns

    b_o = np.asarray(b_o, dtype=np.float32)
    out = np.empty((B, L, D), np.float32)
    for b in range(B):
        # out dram is [128, 16, 1024]: row p, tile lt -> token lt*128+p
        acc = (res.results[2 * b]["out"] + res.results[2 * b + 1]["out"])
        out[b] = acc.reshape(P, MT, D).transpose(1, 0, 2).reshape(L, D) + b_o
    return out


# revision 49
# speedup vs baseline: 1.0004x; 1.0004x over previous
"""Multi-head attention (b=4, l=2048, d=1024, 16 heads) on 8 TRN2 NeuronCores.

Sharding: data parallel over the 4 batches x tensor parallel over 2 head
groups (8 heads each). core = 2*batch + head_group. Each core computes its
batch's attention for its 8 heads plus the partial W_o projection
(row-parallel); the host sums the two partials per batch and adds b_o.
No on-chip collectives needed.

Per-core layout (everything transposed so features sit on partitions):
  xT  [1024, 2048]; Q^T,K^T [512, 2048]; V in [m, o] layout.
  Per head pair (sharing an o-partition tile, rows 0-63 / 64-127):
    S^T = K Q^T for both heads into one [128, 1024] PSUM tile
    -> one exp on ScalarE -> P^T bf16
    -> A^T accumulated via V_aug (64 V cols + ones col -> denom in row 64)
  Softmax denominators are normalized immediately per (pair, lc, head):
  reciprocal straight off PSUM row 64, gpsimd partition-broadcast, one DVE
  multiply -- no DMA round trips and no end-of-pair burst.
  Y_partial = A^T.T-contract @ WoT -> [2048, 1024] fp32.

All DRAM operands are host-pretiled so every DMA is contiguous per
partition (W_q/W_k additionally ot-major so the first projection group's
slab is one small contiguous load); startup DMAs are split fine-grained
across the SP/ACT/Pool queues in first-use order so the first matmul
starts ~2.4us in.

The m-tile loop is software-pipelined: S^T for unit i+1 is emitted before
the exp-dependent A^T-accumulation of unit i, so the in-order PE queue
always holds exp-independent work while ScalarE runs.  Filler work (the
next pair's QK projection groups, the O-projection tiles) is spread one
or two matmuls per m-tile slot inside the attention stream -- whole-group
bursts only in pair 0, whose own groups have just-in-time deadlines; the
last chunk normalizes in 128-column slices so the four tail O-projections
unlock one by one.  Everything is bf16/f32: on this hardware fp8
DoubleRow costs the same per instruction as bf16 (2x MACs only via
K-packing), and any fp8 quantization in the value path adds ~2.5% RMS to
the output -- over the error budget -- so the bf16 stream-cycle floor
(~327us busy PE) is the binding constraint and scheduling is everything.
"""

import os

import numpy as np

B = 4
L = 2048
D = 1024
P = 128
NH = 8          # heads per core
DH = 64
O = NH * DH     # 512 qkv dims per core
DT = D // P     # 8 d-tiles
OT = O // P     # 4 o-tiles (= head pairs)
MT = L // P     # 16 m-tiles
LC4 = L // 512  # 4 l-chunks of 512

_cache = {}


def _build():
    import concourse.tile as tile
    from concourse import bacc, mybir

    nc = bacc.Bacc("TRN2", target_bir_lowering=False, debug=False)
    bf = mybir.dt.bfloat16
    f32 = mybir.dt.float32
    Exp = mybir.ActivationFunctionType.Exp

    # host-pretiled: row p holds the concatenation over tiles (see
    # make_in_maps) so per-partition DRAM runs are large and contiguous.
    xT = nc.dram_tensor("xT", [P, DT * L], bf, kind="ExternalInput").ap()
    # wq/wk are host-pretiled ot-major ([p, ot, dt, 128]) so the ot=0 slab
    # the first projection group needs is one contiguous 2KB-per-partition
    # DMA; wv/wo stay dt-major like the baseline.
    wqT = nc.dram_tensor("wqT", [P, DT * O], bf, kind="ExternalInput").ap()
    wkT = nc.dram_tensor("wkT", [P, DT * O], bf, kind="ExternalInput").ap()
    wvT = nc.dram_tensor("wvT", [P, DT * O], bf, kind="ExternalInput").ap()
    woT = nc.dram_tensor("woT", [P, OT * D], bf, kind="ExternalInput").ap()
    out = nc.dram_tensor("out", [P, MT * D], f32, kind="ExternalOutput").ap()
    xT3 = xT.rearrange("p (dt l) -> p dt l", l=L)
    wq4 = wqT.rearrange("p (ot dt c) -> p ot dt c", dt=DT, c=P)
    wk4 = wkT.rearrange("p (ot dt c) -> p ot dt c", dt=DT, c=P)
    out3 = out.rearrange("p (lt j) -> p lt j", j=D)

    with tile.TileContext(nc) as tc:
        with (
            tc.tile_pool(name="persist", bufs=1) as pp,
            tc.tile_pool(name="work", bufs=3) as wp,
            tc.tile_pool(name="psum", bufs=1, space="PSUM") as psp,
        ):
            xT_sb = pp.tile([P, DT, L], bf, name="xT_sb")
            wq_sb = pp.tile([P, OT, DT, P], bf, name="wq_sb")
            wk_sb = pp.tile([P, OT, DT, P], bf, name="wk_sb")
            wv_sb = pp.tile([P, DT, O], bf, name="wv_sb")
            wo_sb = pp.tile([P, OT, D], bf, name="wo_sb")
            qT_sb = pp.tile([P, OT, L], bf, name="qT_sb")
            kT_sb = pp.tile([P, OT, L], bf, name="kT_sb")
            vaug_sb = pp.tile([P, MT, NH * 65], bf, name="vaug_sb")
            at_sb = pp.tile([P, OT, L], bf, name="at_sb")

            # Fine-grained startup: the ot=0 slices of W_q/W_k plus the
            # first 512 columns of x are all the first two projection
            # groups need; load those first, then stream the rest in
            # first-use order across the three DMA-capable queues (SP,
            # Pool, ACT) so nothing serializes behind a bulk load.
            nc.gpsimd.dma_start(xT_sb[:, 0:4, 0:512], xT3[:, 0:4, 0:512])
            nc.scalar.dma_start(xT_sb[:, 4:8, 0:512], xT3[:, 4:8, 0:512])
            nc.sync.dma_start(wq_sb[:, 0, 0:4], wq4[:, 0, 0:4])
            nc.sync.dma_start(wq_sb[:, 0, 4:8], wq4[:, 0, 4:8])
            nc.sync.dma_start(wk_sb[:, 0], wk4[:, 0])
            nc.scalar.dma_start(xT_sb[:, :, 512:1024], xT3[:, :, 512:1024])
            nc.sync.dma_start(wv_sb[:], wvT[:].rearrange("p (dt o) -> p dt o", o=O))
            nc.sync.dma_start(wq_sb[:, 1:OT], wq4[:, 1:OT])
            nc.sync.dma_start(wk_sb[:, 1:OT], wk4[:, 1:OT])
            nc.gpsimd.dma_start(xT_sb[:, :, 1024:1536], xT3[:, :, 1024:1536])
            nc.gpsimd.dma_start(xT_sb[:, :, 1536:2048], xT3[:, :, 1536:2048])
            nc.sync.dma_start(wo_sb[:], woT[:].rearrange("p (ot j) -> p ot j", j=D))

            # ones columns for the softmax-denominator trick; V copies below
            # fill the first 64 columns of each head's 65-column block, so
            # only column 64 of each block actually needs the memset.
            for h in range(NH):
                nc.gpsimd.memset(vaug_sb[:, :, h * 65 + 64], 1.0)

            def proj_qk_group(w_sb, dst, ot, lc):
                ps = psp.tile([P, 512], f32, tag="mm512", bufs=2, name="ps_qk")
                for dt in range(DT):
                    nc.tensor.matmul(
                        ps[:],
                        w_sb[:, ot, dt, :],
                        xT_sb[:, dt, lc * 512:(lc + 1) * 512],
                        start=(dt == 0),
                        stop=(dt == DT - 1),
                    )
                nc.vector.tensor_copy(dst[:, ot, lc * 512:(lc + 1) * 512], ps[:])

            def proj_v(mt):
                ps = psp.tile([P, 512], f32, tag="mm512", bufs=2, name="ps_v")
                for dt in range(DT):
                    nc.tensor.matmul(
                        ps[:],
                        xT_sb[:, dt, mt * P:(mt + 1) * P],
                        wv_sb[:, dt, :],
                        start=(dt == 0),
                        stop=(dt == DT - 1),
                    )
                for h in range(NH):
                    nc.gpsimd.tensor_copy(
                        vaug_sb[:, mt, h * 65:h * 65 + 64],
                        ps[:, h * DH:(h + 1) * DH],
                    )

            def proj_o(lt, tail=False):
                ob = wp.tile([P, 1024], f32, tag="ob", bufs=3, name="ob")
                for jc in range(2):
                    ps = psp.tile([P, 512], f32, tag="mm512", bufs=2, name="ps_o")
                    for ot in range(OT):
                        nc.tensor.matmul(
                            ps[:],
                            at_sb[:, ot, lt * P:(lt + 1) * P],
                            wo_sb[:, ot, jc * 512:(jc + 1) * 512],
                            start=(ot == 0),
                            stop=(ot == OT - 1),
                        )
                    nc.gpsimd.tensor_copy(ob[:, jc * 512:(jc + 1) * 512], ps[:])
                    if tail:
                        # drain in quarters on both spare queues as soon as
                        # each copy lands, instead of one big store.
                        for q, eng in ((0, nc.sync), (1, nc.gpsimd)):
                            qs = slice(jc * 512 + q * 256, jc * 512 + (q + 1) * 256)
                            eng.dma_start(out3[:, lt, qs], ob[:, qs])
                if not tail:
                    nc.sync.dma_start(out3[:, lt, :], ob[:])

            # Only the two groups the very first S^T matmul needs are
            # emitted upfront; everything else streams in as filler work
            # inside the attention mt loops.
            proj_qk_group(wq_sb, qT_sb, 0, 0)
            proj_qk_group(wk_sb, kT_sb, 0, 0)

            # ---- attention, one head pair at a time -----------------------
            # Filler emissions (projection groups / O-projection tiles) are
            # woven INSIDE the mt loops so the PE always has matmul work
            # during the ACT-bound exp stream.  Pair 0 uses whole-group
            # bursts (its own groups have hard deadlines and mixing bursts
            # with spread streams on one PSUM tag would deadlock the
            # in-order PE queue); pairs 1-3 spread their filler matmuls one
            # or two per m-tile so the PE never drains between bursts.
            def group_thunks(w_sb, dst, ot, lc):
                st = {}

                def mm(dt):
                    if dt == 0:
                        st["ps"] = psp.tile(
                            [P, 512], f32, tag="mm512", bufs=2, name="ps_qk"
                        )
                    nc.tensor.matmul(
                        st["ps"][:],
                        w_sb[:, ot, dt, :],
                        xT_sb[:, dt, lc * 512:(lc + 1) * 512],
                        start=(dt == 0),
                        stop=(dt == DT - 1),
                    )
                    if dt == DT - 1:
                        nc.vector.tensor_copy(
                            dst[:, ot, lc * 512:(lc + 1) * 512], st["ps"][:]
                        )

                return [lambda dt=dt: mm(dt) for dt in range(DT)]

            def proj_o_thunks(lt):
                st = {}

                def mm(j):
                    jc, ot = divmod(j, OT)
                    if j == 0:
                        st["ob"] = wp.tile([P, 1024], f32, tag="ob", bufs=3, name="ob")
                    if ot == 0:
                        st["ps"] = psp.tile(
                            [P, 512], f32, tag="mm512", bufs=2, name="ps_o"
                        )
                    nc.tensor.matmul(
                        st["ps"][:],
                        at_sb[:, ot, lt * P:(lt + 1) * P],
                        wo_sb[:, ot, jc * 512:(jc + 1) * 512],
                        start=(ot == 0),
                        stop=(ot == OT - 1),
                    )
                    if ot == OT - 1:
                        nc.vector.tensor_copy(
                            st["ob"][:, jc * 512:(jc + 1) * 512], st["ps"][:]
                        )
                        if jc == 1:
                            nc.sync.dma_start(out3[:, lt, :], st["ob"][:])

                return [lambda j=j: mm(j) for j in range(2 * OT)]

            def fillers_for(pair, lc):
                f = {}
                if pair == 0:
                    # pair 0's own remaining groups: kT m-groups just in
                    # time (S at m-tile mt reads kT group mt//4), qT for
                    # the next l-chunk midway through the previous one.
                    if lc == 0:
                        # S for m-tile 4g is emitted one unit early (software
                        # pipelining), so group g must be emitted by slot
                        # 4g-2, not 4g-1.
                        for g in (1, 2, 3):
                            f.setdefault(4 * g - 2, []).append(
                                lambda g=g: proj_qk_group(wk_sb, kT_sb, 0, g))
                    if lc < 3:
                        f.setdefault(6, []).append(
                            lambda lc=lc: proj_qk_group(wq_sb, qT_sb, 0, lc + 1))
                stream = []
                if pair < OT - 1:
                    # next pair's 8 groups: two per l-chunk, k-groups first
                    nxt = pair + 1
                    order = [(wq_sb, qT_sb, 0), (wk_sb, kT_sb, 0),
                             (wk_sb, kT_sb, 1), (wk_sb, kT_sb, 2),
                             (wk_sb, kT_sb, 3), (wq_sb, qT_sb, 1),
                             (wq_sb, qT_sb, 2), (wq_sb, qT_sb, 3)]
                    for i in (2 * lc, 2 * lc + 1):
                        w_sb, dst, g = order[i]
                        if pair == 0:
                            f.setdefault(7 + 8 * (i % 2), []).append(
                                lambda w_sb=w_sb, dst=dst, g=g:
                                    proj_qk_group(w_sb, dst, nxt, g))
                        else:
                            stream.extend(group_thunks(w_sb, dst, nxt, g))
                if pair == OT - 1 and lc > 0:
                    # O-projection tiles unlocked one l-chunk back (the
                    # immediate normalize finishes each chunk's at rows as
                    # the chunk ends).  In the last chunk, hold one back to
                    # bridge the final normalize latency.
                    lts = range(4 * (lc - 1), 4 * lc - (1 if lc == LC4 - 1 else 0))
                    for lt in lts:
                        stream.extend(proj_o_thunks(lt))
                # distribute over slots 0..14 so the final group copy is
                # off the DVE queue before mt15's normalize chain needs it
                k = len(stream)
                n = MT - 1
                for i in range(n):
                    for th in stream[(k * i) // n:(k * (i + 1)) // n]:
                        f.setdefault(i, []).append(th)
                return f

            def normalize(pair, lc, av, rb, sl=None):
                """at[rb:rb+64, pair, lc] = av[0:64] * (1/av[64]) immediately
                off PSUM: DVE reciprocal -> Pool broadcast -> DVE multiply.
                sl selects a 128-column slice (tail pipelining); None = all."""
                if sl is None:
                    csl = slice(0, 512)
                    w = 512
                else:
                    csl = slice(sl * P, (sl + 1) * P)
                    w = P
                cols = slice(lc * 512 + csl.start, lc * 512 + csl.stop)
                rt = wp.tile([1, 512], f32, tag="rt", bufs=4, name="rt")
                nc.vector.reciprocal(rt[:, 0:w], av[64:65, csl])
                rbb = wp.tile([DH, 512], f32, tag="rbb", bufs=4, name="rbb")
                nc.gpsimd.partition_broadcast(rbb[:, 0:w], rt[:, 0:w])
                nc.vector.tensor_mul(
                    out=at_sb[rb:rb + DH, pair, cols],
                    in0=av[0:DH, csl],
                    in1=rbb[:, 0:w],
                )

            # The mt loop is software-pipelined: S^T for unit i+1 is
            # emitted BEFORE the exp-dependent AV of unit i, so the in-order
            # PE queue always has exp-independent work while ACT runs.
            units = [
                (pair, lc, mt)
                for pair in range(OT) for lc in range(LC4) for mt in range(MT)
            ]
            s_tiles = {}

            def emit_S(pair, lc, mt):
                cols = slice(lc * 512, (lc + 1) * 512)
                mcols = slice(mt * P, (mt + 1) * P)
                s = psp.tile([P, 1024], f32, tag="s", bufs=2, name="s")
                s_tiles[(pair, lc, mt)] = s
                nc.tensor.matmul(
                    s[:, 0:512],
                    kT_sb[0:DH, pair, mcols],
                    qT_sb[0:DH, pair, cols],
                    start=True, stop=True,
                )
                nc.tensor.matmul(
                    s[:, 512:1024],
                    kT_sb[DH:2 * DH, pair, mcols],
                    qT_sb[DH:2 * DH, pair, cols],
                    start=True, stop=True,
                )

            emit_S(*units[0])
            fill = {}
            av0 = av1 = None
            for ui, (pair, lc, mt) in enumerate(units):
                h0, h1 = 2 * pair, 2 * pair + 1
                if mt == 0:
                    fill = fillers_for(pair, lc)
                    av0 = psp.tile([P, 512], f32, tag="av", bufs=2, name="av0")
                    av1 = psp.tile([P, 512], f32, tag="av", bufs=2, name="av1")
                s = s_tiles.pop((pair, lc, mt))
                p = wp.tile([P, 1024], bf, tag="p", bufs=8, name="p")
                nc.scalar.activation(p[:], s[:], Exp, scale=0.125)
                if ui + 1 < len(units):
                    emit_S(*units[ui + 1])
                if pair == 0 and lc == 0:
                    proj_v(mt)
                nc.tensor.matmul(
                    av0[0:65, :],
                    vaug_sb[:, mt, h0 * 65:h0 * 65 + 65],
                    p[:, 0:512],
                    start=(mt == 0), stop=(mt == MT - 1),
                )
                nc.tensor.matmul(
                    av1[0:65, :],
                    vaug_sb[:, mt, h1 * 65:h1 * 65 + 65],
                    p[:, 512:1024],
                    start=(mt == 0), stop=(mt == MT - 1),
                )
                for fn in fill.get(mt, ()):
                    fn()
                if mt < MT - 1:
                    continue
                if pair == OT - 1 and lc == LC4 - 1:
                    # Final chunk: normalize in 128-column slices so each
                    # tail O-projection's at columns unlock as early as
                    # possible; proj_o(11) (needing only lc2's at) keeps
                    # the PE fed during the first slice's chain.
                    normalize(pair, lc, av0, 0, sl=0)
                    normalize(pair, lc, av1, DH, sl=0)
                    proj_o(11)
                    for sl in range(1, 4):
                        normalize(pair, lc, av0, 0, sl=sl)
                        normalize(pair, lc, av1, DH, sl=sl)
                        proj_o(11 + sl, tail=True)
                else:
                    normalize(pair, lc, av0, 0)
                    normalize(pair, lc, av1, DH)
            proj_o(15, tail=True)

    nc.compile()
    return nc


def get_nc():
    if "nc" not in _cache:
        _cache["nc"] = _build()
    return _cache["nc"]


def _pretile(a, p=P):
    """[T*p, F] -> [p, T*F] with row i holding concat over tiles t of a[t*p+i]."""
    t = a.shape[0] // p
    return np.ascontiguousarray(
        a.reshape(t, p, a.shape[1]).transpose(1, 0, 2).reshape(p, t * a.shape[1])
    )


def _pretile_ot(a, p=P):
    """[DT*p, OT*p] -> [p, OT*DT*p]: ot-major pretile so each ot's slab of
    W^T ([p partitions, DT, p] columns) is contiguous per partition."""
    dt = a.shape[0] // p
    ot = a.shape[1] // p
    # a[dt*p + i, ot*p + c] -> out[i, ((ot*DT + dt)*p) + c]
    return np.ascontiguousarray(
        a.reshape(dt, p, ot, p).transpose(1, 2, 0, 3).reshape(p, ot * dt * p)
    )


def make_in_maps(x, W_q, W_k, W_v, W_o):
    import ml_dtypes

    bf = ml_dtypes.bfloat16
    x = np.asarray(x, dtype=np.float32)
    W_q = np.asarray(W_q, dtype=np.float32)
    W_k = np.asarray(W_k, dtype=np.float32)
    W_v = np.asarray(W_v, dtype=np.float32)
    W_o = np.asarray(W_o, dtype=np.float32)

    in_maps = []
    for core in range(8):
        b, hg = divmod(core, 2)
        rows = slice(hg * O, (hg + 1) * O)
        in_maps.append({
            "xT": _pretile(np.ascontiguousarray(x[b].T)).astype(bf),
            "wqT": _pretile_ot(np.ascontiguousarray(W_q[rows].T)).astype(bf),
            "wkT": _pretile_ot(np.ascontiguousarray(W_k[rows].T)).astype(bf),
            "wvT": _pretile(np.ascontiguousarray(W_v[rows].T)).astype(bf),
            "woT": _pretile(np.ascontiguousarray(W_o[:, rows].T)).astype(bf),
        })
    return in_maps


def kernel(x, W_q, W_k, W_v, W_o, b_o):
    from concourse.bass_utils import run_bass_kernel_spmd

    nc = get_nc()
    in_maps = make_in_maps(x, W_q, W_k, W_v, W_o)
    trace = bool(int(os.environ.get("ATTN_TRACE", "0")))
    try:
        res = run_bass_kernel_spmd(nc, in_maps, core_ids=list(range(8)), trace=trace)
    except ModuleNotFoundError:
        # this container lacks the axon NTFF profile hook; rerun untraced
        trace = False
        res = run_bass_kernel_spmd(nc, in_maps, core_ids=list(range(8)), trace=False)
    if trace and res.exec_time_ns is not None:
        _cache["exec_time_ns"] = res.exec_time_ns
        _cache["mean_exec_time_ns"] = res.mean_exec_time_ns

    b_o = np.asarray(b_o, dtype=np.float32)
    out = np.empty((B, L, D), np.float32)
    for b in range(B):
        # out dram is [128, 16, 1024]: row p, tile lt -> token lt*128+p
        acc = (res.results[2 * b]["out"] + res.results[2 * b + 1]["out"])
        out[b] = acc.reshape(P, MT, D).transpose(1, 0, 2).reshape(L, D) + b_o
    return out
